# revision 11
# baseline (speedup 1.0000x reference)
"""Single-head causal attention (CustomHead) on 8 Trainium2 NeuronCores.

Reference (per batch b):
    q = x Wq^T ; k = x Wk^T ; v = x Wv^T          (x: [T, C], W*: [H, C])
    S = q k^T * C**-0.5 ; causal mask ; softmax ; out = P v    ([T, H])

Sharding: data-parallel over batch B=32 across 8 cores (4 batches/core).
Each core holds full Wq/Wk/Wv.

Kernel plan per core (T=2048, C=1024, H=128), fp32 accum everywhere:
  - x loaded with SWDGE cast-DMA (fp32->bf16 in the DMA engine).
  - x^T produced half by PE transposes (chunks 0-3, PSUM->SBUF via DVE)
    and half by DMA XBAR transposes (chunks 4-7, direct to SBUF) to
    split the transpose cost between idle engines.
  - x^T cast to fp8(e4m3) on DVE; q^T/k^T use fp8 DoubleRow matmuls
    (K=256/instr, ~1.5x PE throughput).  Wq/Wk pre-scaled by 16 so the
    0.02-std weights sit in fp8 normal range; the 256x score scale is
    folded into the exp activation scale.  v stays bf16 end-to-end
    (fp8 anywhere in the v path fails the accuracy budget).
  - v^T -> natural [s, h] + ones column via one XBAR transpose per
    batch into a strided [128, 16, 144] tile (ones col at 128 gives
    the softmax denominator for free in the P.V matmuls).
  - Scores computed transposed: S^T[s, t] = kT(s-block) vs qT, 512-wide
    PSUM chunks, one exp (ACT) per chunk; no max-subtraction (scores
    are bounded, exp is safe in fp32).  P^T rows stored bf16 in
    right-sized tiles ([128, 2048-128*ss]).
  - Causal handling: S^T block-row ss only computes t >= 512*(ss//4);
    the diagonal 128x128 block is masked by an upper-triangular 0/1
    multiply after exp.
  - The whole batch loop is software-pipelined: batch b+1's
    transpose/cast/projection work is emitted in program order between
    batch b's score-row S matmuls and P.V matmuls, so the PE FIFO has
    useful work while P.V waits on the ACT exp chain.
"""

import numpy as np

B, T, C, H = 32, 2048, 1024, 128
NCORES = 8
BL = B // NCORES  # batches per core

_CACHE = {}


def _build():
    import concourse.bass as bass
    import concourse.tile as tile
    from concourse import bacc, mybir
    from concourse.masks import make_identity, make_upper_triangular

    f32 = mybir.dt.float32
    bf16 = mybir.dt.bfloat16
    f8 = mybir.dt.float8e4
    Exp = mybir.ActivationFunctionType.Exp
    DR = mybir.MatmulPerfMode.DoubleRow
    WS = 16.0  # fp8 pre-scale for Wq/Wk
    EXPSCALE = float(C) ** -0.5 / (WS * WS)

    nc = bacc.Bacc(
        "TRN2",
        target_bir_lowering=False,
        debug=False,
        enable_asserts=False,
        num_devices=NCORES,
    )
    x_ap = nc.dram_tensor("x", [BL, T, C], f32, kind="ExternalInput").ap()
    wk_ap = nc.dram_tensor("Wk", [H, C], f32, kind="ExternalInput").ap()
    wq_ap = nc.dram_tensor("Wq", [H, C], f32, kind="ExternalInput").ap()
    wv_ap = nc.dram_tensor("Wv", [H, C], f32, kind="ExternalInput").ap()
    out_ap = nc.dram_tensor("out", [BL, T, H], f32, kind="ExternalOutput").ap()

    with tile.TileContext(nc) as tc:
        from contextlib import ExitStack

        with ExitStack() as ctx:
            consts = ctx.enter_context(tc.tile_pool(name="consts", bufs=1))
            wstage = ctx.enter_context(tc.tile_pool(name="wstage", bufs=1))
            xbf_p = ctx.enter_context(tc.tile_pool(name="xbf", bufs=8))
            xt_p = ctx.enter_context(tc.tile_pool(name="xt", bufs=2))
            x8_p = ctx.enter_context(tc.tile_pool(name="x8", bufs=1))
            qk_p = ctx.enter_context(tc.tile_pool(name="qk", bufs=2))
            va_p = ctx.enter_context(tc.tile_pool(name="va", bufs=2))
            pr_p = ctx.enter_context(tc.tile_pool(name="prow", bufs=1))
            osb_p = ctx.enter_context(tc.tile_pool(name="osb", bufs=2))
            rc_p = ctx.enter_context(tc.tile_pool(name="rc", bufs=4))
            trans_ps = ctx.enter_context(
                tc.tile_pool(name="trans_ps", bufs=2, space="PSUM")
            )
            mm_ps = ctx.enter_context(tc.tile_pool(name="mm_ps", bufs=2, space="PSUM"))
            srow_ps = ctx.enter_context(
                tc.tile_pool(name="srow_ps", bufs=2, space="PSUM")
            )
            pv_ps = ctx.enter_context(tc.tile_pool(name="pv_ps", bufs=2, space="PSUM"))

            ident = consts.tile([128, 128], bf16)
            make_identity(nc, ident)

            # trimask[s, t] = 1 if s <= t else 0 (valid region of the
            # transposed diagonal block)
            trimask = consts.tile([128, 128], bf16)
            make_upper_triangular(nc, trimask, val=1.0, diag=True)

            # --- weights: load, scale+cast, XBAR-transpose to [c%128, cc, h] ---
            W8 = {}
            for name, wap in (("q", wq_ap), ("k", wk_ap)):
                wnat = wstage.tile([128, C], f32, tag="wnat")
                nc.sync.dma_start(out=wnat, in_=wap)
                wbf = wstage.tile([128, C], bf16, tag="wbf")
                nc.vector.tensor_scalar_mul(wbf, wnat, WS)
                wt3 = wstage.tile([128, 8, 128], bf16, tag="wt3")
                nc.sync.dma_start(out=wt3, in_=wbf, transpose=True)
                w8 = consts.tile([128, 8, 128], f8, tag=f"w8{name}", name=f"w8{name}")
                nc.vector.tensor_copy(out=w8, in_=wt3)
                W8[name] = w8
            wnat = wstage.tile([128, C], f32, tag="wnat")
            nc.sync.dma_start(out=wnat, in_=wv_ap)
            wbf = wstage.tile([128, C], bf16, tag="wbf")
            nc.vector.tensor_copy(out=wbf, in_=wnat)
            wtv = consts.tile([128, 8, 128], bf16)
            nc.sync.dma_start(out=wtv, in_=wbf, transpose=True)

            def emit_loads(b):
                xbfs = []
                for tt in range(16):
                    xb = xbf_p.tile([128, C], bf16, tag="xb", name=f"xb{tt}")
                    nc.gpsimd.dma_start(
                        out=xb, in_=x_ap[b, 128 * tt : 128 * (tt + 1), :]
                    )
                    xbfs.append(xb)
                return xbfs

            def make_b_units(b, xbfs):
                """Per-batch transpose/cast/projection work as a list of
                closures; each is one PE-queue-sized unit."""
                # allocate tiles up front (allocation order = program order
                # only matters for pool slot cycling, not scheduling)
                xt_lo = xt_p.tile([128, 4, T], bf16, tag="xt_lo", name="xt_lo")
                xt_hi = xt_p.tile([128, 4, T], bf16, tag="xt_hi", name="xt_hi")
                x8 = x8_p.tile([128, 8, T], f8)
                qT = qk_p.tile([128, T], bf16, tag="qT")
                kT = qk_p.tile([128, T], bf16, tag="kT")
                vT = qk_p.tile([128, T], bf16, tag="vT")
                va = va_p.tile([128, 16, 144], bf16)

                def xt_sl(cc, sl):
                    return (xt_lo if cc < 4 else xt_hi)[:, cc % 4, sl]

                units = []

                def xbar_unit(tt8):
                    def f():
                        for tt in range(8 * tt8, 8 * tt8 + 8):
                            nc.sync.dma_start(
                                out=xt_hi[:, :, 128 * tt : 128 * (tt + 1)],
                                in_=xbfs[tt][:, 512:1024],
                                transpose=True,
                            )
                    return f

                def trans_unit(tt8, cc):
                    def f():
                        ps = trans_ps.tile([128, 1024], bf16)
                        for m in range(8):
                            nc.tensor.transpose(
                                ps[:, 128 * m : 128 * (m + 1)],
                                xbfs[8 * tt8 + m][:, 128 * cc : 128 * (cc + 1)],
                                ident,
                            )
                        nc.vector.tensor_copy(
                            out=xt_lo[:, cc, 1024 * tt8 : 1024 * (tt8 + 1)], in_=ps
                        )
                    return f

                def cast_unit(tt8, half):
                    def f():
                        sl = slice(1024 * tt8, 1024 * (tt8 + 1))
                        nc.vector.tensor_copy(
                            out=x8[:, 4 * half : 4 * half + 4, sl],
                            in_=(xt_lo if half == 0 else xt_hi)[:, :, sl],
                        )
                    return f

                def qk_unit(w8, dst, s4):
                    def f():
                        ps = mm_ps.tile([128, 512], f32)
                        for g in range(4):
                            nc.tensor.matmul(
                                ps,
                                w8[:, 2 * g : 2 * g + 2, :],
                                x8[:, 2 * g : 2 * g + 2, 512 * s4 : 512 * (s4 + 1)],
                                start=(g == 0),
                                stop=(g == 3),
                                perf_mode=DR,
                            )
                        nc.scalar.copy(out=dst[:, 512 * s4 : 512 * (s4 + 1)], in_=ps)
                    return f

                def v_unit(s4):
                    def f():
                        ps = mm_ps.tile([128, 512], f32)
                        for cc in range(8):
                            nc.tensor.matmul(
                                ps,
                                wtv[:, cc, :],
                                xt_sl(cc, slice(512 * s4, 512 * (s4 + 1))),
                                start=(cc == 0),
                                stop=(cc == 7),
                            )
                        nc.scalar.copy(out=vT[:, 512 * s4 : 512 * (s4 + 1)], in_=ps)
                    return f

                def va_unit():
                    def f():
                        nc.gpsimd.memset(va[:, :, 128:129], 1.0)
                        nc.sync.dma_start(
                            out=va[:, :, 0:128], in_=vT, transpose=True
                        )
                    return f

                for tt8 in range(2):
                    units.append(xbar_unit(tt8))
                    for cc in range(4):
                        units.append(trans_unit(tt8, cc))
                    units.append(cast_unit(tt8, 0))
                    units.append(cast_unit(tt8, 1))
                    for s4 in (2 * tt8, 2 * tt8 + 1):
                        units.append(qk_unit(W8["q"], qT, s4))
                        units.append(qk_unit(W8["k"], kT, s4))
                        units.append(v_unit(s4))
                units.append(va_unit())
                state = dict(qT=qT, kT=kT, va=va)
                return units, state

            def emit_units(units, n):
                for _ in range(n):
                    if units:
                        units.pop(0)()

            def emit_scores(b, st, next_units):
                qT, kT, va = st["qT"], st["kT"], st["va"]
                out_sb = osb_p.tile([128, 16 * H], f32)
                prows = []
                for ss in range(16):
                    pb = 128 * ss
                    pr = pr_p.tile(
                        [128, T - pb], bf16, tag=f"pr{ss}", name=f"pr{ss}"
                    )
                    prows.append(pr)
                    for tq in range(ss // 4, 4):
                        c0 = 512 * tq
                        x0 = max(pb, c0)  # first causal-needed column
                        d0 = x0 - c0
                        sh = srow_ps.tile([128, 512], f32)
                        nc.tensor.matmul(
                            sh[:, d0:512],
                            kT[:, pb : pb + 128],
                            qT[:, x0 : c0 + 512],
                            start=True,
                            stop=True,
                        )
                        nc.scalar.activation(
                            out=pr[:, x0 - pb : c0 + 512 - pb],
                            in_=sh[:, d0:512],
                            func=Exp,
                            scale=EXPSCALE,
                        )
                    # fill the PE queue with next-batch work while the ACT
                    # exp chain for this row drains (skip early rows: the
                    # next batch's x tiles are still loading)
                    if ss >= 4:
                        emit_units(next_units, 3)
                    nc.vector.tensor_mul(pr[:, 0:128], pr[:, 0:128], trimask)
                    pv = pv_ps.tile([128, H + 1], f32)
                    for j in range(ss + 1):
                        nc.tensor.matmul(
                            pv,
                            prows[j][:, pb - 128 * j : pb - 128 * j + 128],
                            va[:, j, 0 : H + 1],
                            start=(j == 0),
                            stop=(j == ss),
                        )
                    rc = rc_p.tile([128, 1], f32)
                    nc.vector.reciprocal(rc, pv[:, 128:129])
                    nc.vector.tensor_mul(
                        out_sb[:, H * ss : H * (ss + 1)],
                        pv[:, 0:128],
                        rc.broadcast_to([128, H]),
                    )
                emit_units(next_units, len(next_units))
                # out_sb[p, (g h)] -> out[b, 128g+p, h]; split DMAs so the
                # final transfer after the last normalize is small
                np_split = 4 if b == BL - 1 else 2
                npc = 2048 // np_split
                for hh in range(np_split):
                    nc.sync.dma_start(
                        out=out_ap[b, npc * hh : npc * (hh + 1), :].rearrange(
                            "(g p) h -> p g h", p=128
                        ),
                        in_=out_sb[
                            :, npc // 128 * H * hh : npc // 128 * H * (hh + 1)
                        ].rearrange("p (g h) -> p g h", h=H),
                    )

            # --- software-pipelined batch loop ---
            xbfs = emit_loads(0)
            units, st = make_b_units(0, xbfs)
            emit_units(units, len(units))  # prologue: batch 0 B-phase flat
            for b in range(BL):
                if b + 1 < BL:
                    xbfs = emit_loads(b + 1)
                    next_units, next_st = make_b_units(b + 1, xbfs)
                else:
                    next_units, next_st = [], None
                emit_scores(b, st, next_units)
                st = next_st

    nc.compile()
    return nc


def _get_nc():
    if "nc" not in _CACHE:
        _CACHE["nc"] = _build()
    return _CACHE["nc"]


def kernel(x, Wk, Wq, Wv, _trace=False):
    from concourse.bass_utils import run_bass_kernel_spmd

    x = np.ascontiguousarray(np.asarray(x, dtype=np.float32))
    Wk = np.ascontiguousarray(np.asarray(Wk, dtype=np.float32))
    Wq = np.ascontiguousarray(np.asarray(Wq, dtype=np.float32))
    Wv = np.ascontiguousarray(np.asarray(Wv, dtype=np.float32))
    assert x.shape == (B, T, C)

    nc = _get_nc()
    in_maps = [
        {"x": x[i * BL : (i + 1) * BL], "Wk": Wk, "Wq": Wq, "Wv": Wv}
        for i in range(NCORES)
    ]
    res = run_bass_kernel_spmd(nc, in_maps, list(range(NCORES)), trace=_trace)
    out = np.concatenate([res.results[i]["out"] for i in range(NCORES)], axis=0)
    if _trace:
        _CACHE["last_results"] = res
    return out


# revision 13
# speedup vs baseline: 1.6552x; 1.6552x over previous
"""Single-head causal attention (CustomHead) on 8 Trainium2 NeuronCores.

Reference (per batch b):
    q = x Wq^T ; k = x Wk^T ; v = x Wv^T          (x: [T, C], W*: [H, C])
    S = q k^T * C**-0.5 ; causal mask ; softmax ; out = P v    ([T, H])

Sharding: data-parallel over batch B=32 across 8 cores (4 batches/core).
Each core holds full Wq/Wk/Wv.

Kernel plan per core (T=2048, C=1024, H=128), fp32 accum everywhere:
  - x loaded with SWDGE cast-DMA (fp32->bf16 in the DMA engine).
  - x^T produced half by PE transposes (chunks 0-3, PSUM->SBUF via DVE)
    and half by DMA XBAR transposes (chunks 4-7, direct to SBUF) to
    split the transpose cost between idle engines.
  - x^T cast to fp8(e4m3) on DVE; q^T/k^T use fp8 DoubleRow matmuls
    (K=256/instr, ~1.5x PE throughput).  Wq/Wk pre-scaled by 16 so the
    0.02-std weights sit in fp8 normal range; the 256x score scale is
    folded into the exp activation scale.  v stays bf16 end-to-end
    (fp8 anywhere in the v path fails the accuracy budget).
  - v^T -> natural [s, h] + ones column via one XBAR transpose per
    batch into a strided [128, 16, 144] tile (ones col at 128 gives
    the softmax denominator for free in the P.V matmuls).
  - Scores computed transposed: S^T[s, t] = kT(s-block) vs qT, 512-wide
    PSUM chunks, one exp (ACT) per chunk; no max-subtraction (scores
    are bounded, exp is safe in fp32).  P^T rows stored bf16 in
    right-sized tiles ([128, 2048-128*ss]).
  - Causal handling: S^T block-row ss only computes t >= 512*(ss//4);
    the diagonal 128x128 block is masked by an upper-triangular 0/1
    multiply after exp.
  - The whole batch loop is software-pipelined: batch b+1's
    transpose/cast/projection work is emitted in program order between
    batch b's score-row S matmuls and P.V matmuls, so the PE FIFO has
    useful work while P.V waits on the ACT exp chain.
"""

import numpy as np

B, T, C, H = 32, 2048, 1024, 128
NCORES = 8
BL = B // NCORES  # batches per core

_CACHE = {}


def _build():
    import concourse.bass as bass
    import concourse.tile as tile
    from concourse import bacc, mybir
    from concourse.masks import make_identity, make_upper_triangular

    f32 = mybir.dt.float32
    bf16 = mybir.dt.bfloat16
    f8 = mybir.dt.float8e4
    Exp = mybir.ActivationFunctionType.Exp
    DR = mybir.MatmulPerfMode.DoubleRow
    WS = 16.0  # fp8 pre-scale for Wq/Wk
    EXPSCALE = float(C) ** -0.5 / (WS * WS)

    nc = bacc.Bacc(
        "TRN2",
        target_bir_lowering=False,
        debug=False,
        enable_asserts=False,
        num_devices=NCORES,
    )
    x_ap = nc.dram_tensor("x", [BL, T, C], f32, kind="ExternalInput").ap()
    wk_ap = nc.dram_tensor("Wk", [H, C], f32, kind="ExternalInput").ap()
    wq_ap = nc.dram_tensor("Wq", [H, C], f32, kind="ExternalInput").ap()
    wv_ap = nc.dram_tensor("Wv", [H, C], f32, kind="ExternalInput").ap()
    out_ap = nc.dram_tensor("out", [BL, T, H], f32, kind="ExternalOutput").ap()

    with tile.TileContext(nc) as tc:
        from contextlib import ExitStack

        with ExitStack() as ctx:
            consts = ctx.enter_context(tc.tile_pool(name="consts", bufs=1))
            wstage = ctx.enter_context(tc.tile_pool(name="wstage", bufs=1))
            xbf_p = ctx.enter_context(tc.tile_pool(name="xbf", bufs=10))
            xt_p = ctx.enter_context(tc.tile_pool(name="xt", bufs=9))
            x8_p = ctx.enter_context(tc.tile_pool(name="x8", bufs=1))
            qk_p = ctx.enter_context(tc.tile_pool(name="qk", bufs=2))
            va_p = ctx.enter_context(tc.tile_pool(name="va", bufs=2))
            pr_p = ctx.enter_context(tc.tile_pool(name="prow", bufs=1))
            osb_p = ctx.enter_context(tc.tile_pool(name="osb", bufs=2))
            rc_p = ctx.enter_context(tc.tile_pool(name="rc", bufs=4))
            trans_ps = ctx.enter_context(
                tc.tile_pool(name="trans_ps", bufs=2, space="PSUM")
            )
            mm_ps = ctx.enter_context(tc.tile_pool(name="mm_ps", bufs=2, space="PSUM"))
            srow_ps = ctx.enter_context(
                tc.tile_pool(name="srow_ps", bufs=2, space="PSUM")
            )
            pv_ps = ctx.enter_context(tc.tile_pool(name="pv_ps", bufs=2, space="PSUM"))

            ident = consts.tile([128, 128], bf16)
            make_identity(nc, ident)

            # trimask[s, t] = 1 if s <= t else 0 (valid region of the
            # transposed diagonal block)
            trimask = consts.tile([128, 128], bf16)
            make_upper_triangular(nc, trimask, val=1.0, diag=True)

            # --- weights: load, scale+cast, XBAR-transpose to [c%128, cc, h] ---
            W8 = {}
            for name, wap in (("q", wq_ap), ("k", wk_ap)):
                wnat = wstage.tile([128, C], f32, tag="wnat")
                nc.sync.dma_start(out=wnat, in_=wap)
                wbf = wstage.tile([128, C], bf16, tag="wbf")
                nc.vector.tensor_scalar_mul(wbf, wnat, WS)
                wt3 = wstage.tile([128, 8, 128], bf16, tag="wt3")
                nc.sync.dma_start(out=wt3, in_=wbf, transpose=True)
                w8 = consts.tile([128, 8, 128], f8, tag=f"w8{name}", name=f"w8{name}")
                nc.vector.tensor_copy(out=w8, in_=wt3)
                W8[name] = w8
            wnat = wstage.tile([128, C], f32, tag="wnat")
            nc.sync.dma_start(out=wnat, in_=wv_ap)
            wbf = wstage.tile([128, C], bf16, tag="wbf")
            nc.vector.tensor_copy(out=wbf, in_=wnat)
            wtv = consts.tile([128, 8, 128], bf16)
            nc.sync.dma_start(out=wtv, in_=wbf, transpose=True)

            def emit_loads(b):
                xbfs = []
                for tt in range(16):
                    xb = xbf_p.tile([128, C], bf16, tag="xb", name=f"xb{tt}")
                    nc.gpsimd.dma_start(
                        out=xb, in_=x_ap[b, 128 * tt : 128 * (tt + 1), :]
                    )
                    xbfs.append(xb)
                return xbfs

            def make_b_units(b, xbfs):
                """Per-batch transpose/cast/projection work as a list of
                closures; each is one PE-queue-sized unit."""
                xts = [
                    xt_p.tile([128, T], bf16, name=f"xt{cc}", tag="xt")
                    for cc in range(8)
                ]
                x8 = x8_p.tile([128, 8, T], f8)
                qT = qk_p.tile([128, T], bf16, tag="qT")
                kT = qk_p.tile([128, T], bf16, tag="kT")
                vT = qk_p.tile([128, T], bf16, tag="vT")
                va = va_p.tile([128, 16, 144], bf16)

                units = []

                def trans_unit(tt8, cc):
                    def f():
                        ps = trans_ps.tile([128, 1024], bf16)
                        for m in range(8):
                            nc.tensor.transpose(
                                ps[:, 128 * m : 128 * (m + 1)],
                                xbfs[8 * tt8 + m][:, 128 * cc : 128 * (cc + 1)],
                                ident,
                            )
                        nc.vector.tensor_copy(
                            out=xts[cc][:, 1024 * tt8 : 1024 * (tt8 + 1)], in_=ps
                        )
                    return f

                def cast_unit(tt8, half):
                    def f():
                        sl = slice(1024 * tt8, 1024 * (tt8 + 1))
                        for cc in range(4 * half, 4 * half + 4):
                            nc.vector.tensor_copy(
                                out=x8[:, cc, sl], in_=xts[cc][:, sl]
                            )
                    return f

                def qk_unit(w8, dst, s4):
                    def f():
                        ps = mm_ps.tile([128, 512], f32)
                        for g in range(4):
                            nc.tensor.matmul(
                                ps,
                                w8[:, 2 * g : 2 * g + 2, :],
                                x8[:, 2 * g : 2 * g + 2, 512 * s4 : 512 * (s4 + 1)],
                                start=(g == 0),
                                stop=(g == 3),
                                perf_mode=DR,
                            )
                        nc.scalar.copy(out=dst[:, 512 * s4 : 512 * (s4 + 1)], in_=ps)
                    return f

                def v_unit(s4):
                    def f():
                        ps = mm_ps.tile([128, 512], f32)
                        for cc in range(8):
                            nc.tensor.matmul(
                                ps,
                                wtv[:, cc, :],
                                xts[cc][:, 512 * s4 : 512 * (s4 + 1)],
                                start=(cc == 0),
                                stop=(cc == 7),
                            )
                        nc.scalar.copy(out=vT[:, 512 * s4 : 512 * (s4 + 1)], in_=ps)
                    return f

                def va_unit():
                    def f():
                        nc.gpsimd.memset(va[:, :, 128:129], 1.0)
                        nc.sync.dma_start(
                            out=va[:, :, 0:128], in_=vT, transpose=True
                        )
                    return f

                for tt8 in range(2):
                    for cc in range(8):
                        units.append(trans_unit(tt8, cc))
                    units.append(cast_unit(tt8, 0))
                    units.append(cast_unit(tt8, 1))
                    for s4 in (2 * tt8, 2 * tt8 + 1):
                        units.append(qk_unit(W8["q"], qT, s4))
                        units.append(qk_unit(W8["k"], kT, s4))
                        units.append(v_unit(s4))
                units.append(va_unit())
                state = dict(qT=qT, kT=kT, va=va)
                return units, state

            def emit_units(units, n):
                for _ in range(n):
                    if units:
                        units.pop(0)()

            def emit_scores(b, st, next_units):
                qT, kT, va = st["qT"], st["kT"], st["va"]
                out_sb = osb_p.tile([128, 16 * H], f32)
                prows = []
                for ss in range(16):
                    pb = 128 * ss
                    pr = pr_p.tile(
                        [128, T - pb], bf16, tag=f"pr{ss}", name=f"pr{ss}"
                    )
                    prows.append(pr)
                    for tq in range(ss // 4, 4):
                        c0 = 512 * tq
                        x0 = max(pb, c0)  # first causal-needed column
                        d0 = x0 - c0
                        sh = srow_ps.tile([128, 512], f32)
                        nc.tensor.matmul(
                            sh[:, d0:512],
                            kT[:, pb : pb + 128],
                            qT[:, x0 : c0 + 512],
                            start=True,
                            stop=True,
                        )
                        nc.scalar.activation(
                            out=pr[:, x0 - pb : c0 + 512 - pb],
                            in_=sh[:, d0:512],
                            func=Exp,
                            scale=EXPSCALE,
                        )
                    # fill the PE queue with next-batch work while the ACT
                    # exp chain for this row drains (skip early rows: the
                    # next batch's x tiles are still loading)
                    if ss >= 4:
                        emit_units(next_units, 3)
                    nc.vector.tensor_mul(pr[:, 0:128], pr[:, 0:128], trimask)
                    pv = pv_ps.tile([128, H + 1], f32)
                    for j in range(ss + 1):
                        nc.tensor.matmul(
                            pv,
                            prows[j][:, pb - 128 * j : pb - 128 * j + 128],
                            va[:, j, 0 : H + 1],
                            start=(j == 0),
                            stop=(j == ss),
                        )
                    rc = rc_p.tile([128, 1], f32)
                    nc.vector.reciprocal(rc, pv[:, 128:129])
                    nc.vector.tensor_mul(
                        out_sb[:, H * ss : H * (ss + 1)],
                        pv[:, 0:128],
                        rc.broadcast_to([128, H]),
                    )
                emit_units(next_units, len(next_units))
                # out_sb[p, (g h)] -> out[b, 128g+p, h]; split DMAs so the
                # final transfer after the last normalize is small
                np_split = 4 if b == BL - 1 else 2
                npc = 2048 // np_split
                for hh in range(np_split):
                    nc.sync.dma_start(
                        out=out_ap[b, npc * hh : npc * (hh + 1), :].rearrange(
                            "(g p) h -> p g h", p=128
                        ),
                        in_=out_sb[
                            :, npc // 128 * H * hh : npc // 128 * H * (hh + 1)
                        ].rearrange("p (g h) -> p g h", h=H),
                    )

            # --- software-pipelined batch loop ---
            xbfs = emit_loads(0)
            units, st = make_b_units(0, xbfs)
            emit_units(units, len(units))  # prologue: batch 0 B-phase flat
            for b in range(BL):
                if b + 1 < BL:
                    xbfs = emit_loads(b + 1)
                    next_units, next_st = make_b_units(b + 1, xbfs)
                else:
                    next_units, next_st = [], None
                emit_scores(b, st, next_units)
                st = next_st

    nc.compile()
    return nc


def _get_nc():
    if "nc" not in _CACHE:
        _CACHE["nc"] = _build()
    return _CACHE["nc"]


def kernel(x, Wk, Wq, Wv, _trace=False):
    from concourse.bass_utils import run_bass_kernel_spmd

    x = np.ascontiguousarray(np.asarray(x, dtype=np.float32))
    Wk = np.ascontiguousarray(np.asarray(Wk, dtype=np.float32))
    Wq = np.ascontiguousarray(np.asarray(Wq, dtype=np.float32))
    Wv = np.ascontiguousarray(np.asarray(Wv, dtype=np.float32))
    assert x.shape == (B, T, C)

    nc = _get_nc()
    in_maps = [
        {"x": x[i * BL : (i + 1) * BL], "Wk": Wk, "Wq": Wq, "Wv": Wv}
        for i in range(NCORES)
    ]
    res = run_bass_kernel_spmd(nc, in_maps, list(range(NCORES)), trace=_trace)
    out = np.concatenate([res.results[i]["out"] for i in range(NCORES)], axis=0)
    if _trace:
        _CACHE["last_results"] = res
    return out


# revision 19
# speedup vs baseline: 1.6856x; 1.0184x over previous
"""Single-head causal attention (CustomHead) on 8 Trainium2 NeuronCores.

Reference (per batch b):
    q = x Wq^T ; k = x Wk^T ; v = x Wv^T          (x: [T, C], W*: [H, C])
    S = q k^T * C**-0.5 ; causal mask ; softmax ; out = P v    ([T, H])

Sharding: data-parallel over batch B=32 across 8 cores (4 batches/core).
Each core holds full Wq/Wk/Wv.

Kernel plan per core (T=2048, C=1024, H=128), fp32 accum everywhere:
  - x loaded with SWDGE cast-DMA (fp32->bf16 in the DMA engine).
  - x^T produced half by PE transposes (chunks 0-3, PSUM->SBUF via DVE)
    and half by DMA XBAR transposes (chunks 4-7, direct to SBUF) to
    split the transpose cost between idle engines.
  - x^T cast to fp8(e4m3) on DVE; q^T/k^T use fp8 DoubleRow matmuls
    (K=256/instr, ~1.5x PE throughput).  Wq/Wk pre-scaled by 16 so the
    0.02-std weights sit in fp8 normal range; the 256x score scale is
    folded into the exp activation scale.  v stays bf16 end-to-end
    (fp8 anywhere in the v path fails the accuracy budget).
  - v^T -> natural [s, h] + ones column via one XBAR transpose per
    batch into a strided [128, 16, 144] tile (ones col at 128 gives
    the softmax denominator for free in the P.V matmuls).
  - Scores computed transposed: S^T[s, t] = kT(s-block) vs qT, 512-wide
    PSUM chunks, one exp (ACT) per chunk; no max-subtraction (scores
    are bounded, exp is safe in fp32).  P^T rows stored bf16 in
    right-sized tiles ([128, 2048-128*ss]).
  - Causal handling: S^T block-row ss only computes t >= 512*(ss//4);
    the diagonal 128x128 block is masked by an upper-triangular 0/1
    multiply after exp.
  - The whole batch loop is software-pipelined: batch b+1's
    transpose/cast/projection work is emitted in program order between
    batch b's score-row S matmuls and P.V matmuls, so the PE FIFO has
    useful work while P.V waits on the ACT exp chain.
"""

import numpy as np

B, T, C, H = 32, 2048, 1024, 128
NCORES = 8
BL = B // NCORES  # batches per core

_CACHE = {}


def _build():
    import concourse.bass as bass
    import concourse.tile as tile
    from concourse import bacc, mybir
    from concourse.masks import make_identity, make_upper_triangular

    f32 = mybir.dt.float32
    bf16 = mybir.dt.bfloat16
    f8 = mybir.dt.float8e4
    Exp = mybir.ActivationFunctionType.Exp
    DR = mybir.MatmulPerfMode.DoubleRow
    WS = 16.0  # fp8 pre-scale for Wq/Wk
    EXPSCALE = float(C) ** -0.5 / (WS * WS)

    nc = bacc.Bacc(
        "TRN2",
        target_bir_lowering=False,
        debug=False,
        enable_asserts=False,
        num_devices=NCORES,
    )
    x_ap = nc.dram_tensor("x", [BL, T, C], f32, kind="ExternalInput").ap()
    wk_ap = nc.dram_tensor("Wk", [H, C], f32, kind="ExternalInput").ap()
    wq_ap = nc.dram_tensor("Wq", [H, C], f32, kind="ExternalInput").ap()
    wv_ap = nc.dram_tensor("Wv", [H, C], f32, kind="ExternalInput").ap()
    out_ap = nc.dram_tensor("out", [BL, T, H], f32, kind="ExternalOutput").ap()

    with tile.TileContext(nc) as tc:
        from contextlib import ExitStack

        with ExitStack() as ctx:
            consts = ctx.enter_context(tc.tile_pool(name="consts", bufs=1))
            wstage = ctx.enter_context(tc.tile_pool(name="wstage", bufs=1))
            xbf_p = ctx.enter_context(tc.tile_pool(name="xbf", bufs=10))
            xt_p = ctx.enter_context(tc.tile_pool(name="xt", bufs=9))
            x8_p = ctx.enter_context(tc.tile_pool(name="x8", bufs=1))
            qk_p = ctx.enter_context(tc.tile_pool(name="qk", bufs=2))
            va_p = ctx.enter_context(tc.tile_pool(name="va", bufs=2))
            pr_p = ctx.enter_context(tc.tile_pool(name="prow", bufs=1))
            osb_p = ctx.enter_context(tc.tile_pool(name="osb", bufs=2))
            rc_p = ctx.enter_context(tc.tile_pool(name="rc", bufs=4))
            trans_ps = ctx.enter_context(
                tc.tile_pool(name="trans_ps", bufs=2, space="PSUM")
            )
            mm_ps = ctx.enter_context(tc.tile_pool(name="mm_ps", bufs=2, space="PSUM"))
            srow_ps = ctx.enter_context(
                tc.tile_pool(name="srow_ps", bufs=2, space="PSUM")
            )
            pv_ps = ctx.enter_context(tc.tile_pool(name="pv_ps", bufs=2, space="PSUM"))

            ident = consts.tile([128, 128], bf16)
            make_identity(nc, ident)

            # trimask[s, t] = 1 if s <= t else 0 (valid region of the
            # transposed diagonal block)
            trimask = consts.tile([128, 128], bf16)
            make_upper_triangular(nc, trimask, val=1.0, diag=True)

            # --- weights: load, scale+cast, XBAR-transpose to [c%128, cc, h] ---
            W8 = {}
            for name, wap in (("q", wq_ap), ("k", wk_ap)):
                wnat = wstage.tile([128, C], f32, tag="wnat")
                nc.sync.dma_start(out=wnat, in_=wap)
                wbf = wstage.tile([128, C], bf16, tag="wbf")
                nc.vector.tensor_scalar_mul(wbf, wnat, WS)
                wt3 = wstage.tile([128, 8, 128], bf16, tag="wt3")
                nc.sync.dma_start(out=wt3, in_=wbf, transpose=True)
                w8 = consts.tile([128, 8, 128], f8, tag=f"w8{name}", name=f"w8{name}")
                nc.vector.tensor_copy(out=w8, in_=wt3)
                W8[name] = w8
            wnat = wstage.tile([128, C], f32, tag="wnat")
            nc.sync.dma_start(out=wnat, in_=wv_ap)
            wbf = wstage.tile([128, C], bf16, tag="wbf")
            nc.vector.tensor_copy(out=wbf, in_=wnat)
            wtv = consts.tile([128, 8, 128], bf16)
            nc.sync.dma_start(out=wtv, in_=wbf, transpose=True)

            def emit_loads(b):
                xbfs = []
                for tt in range(16):
                    xb = xbf_p.tile([128, C], bf16, tag="xb", name=f"xb{tt}")
                    nc.gpsimd.dma_start(
                        out=xb, in_=x_ap[b, 128 * tt : 128 * (tt + 1), :]
                    )
                    xbfs.append(xb)
                return xbfs

            def make_b_units(b, xbfs):
                """Per-batch transpose/cast/projection work as a list of
                closures; each is one PE-queue-sized unit."""
                xts = [
                    xt_p.tile([128, T], bf16, name=f"xt{cc}", tag="xt")
                    for cc in range(8)
                ]
                x8 = x8_p.tile([128, 8, T], f8)
                qT = qk_p.tile([128, T], bf16, tag="qT")
                kT = qk_p.tile([128, T], bf16, tag="kT")
                vT = qk_p.tile([128, T], bf16, tag="vT")
                va = va_p.tile([128, 16, 144], bf16)

                units = []

                def trans_unit(tt8, cc):
                    # PE-transpose one [128, 1024] strip of x^T chunk cc,
                    # then immediately DVE-copy (bf16) and cast (fp8) so the
                    # fp8 operand is ready right behind the bf16 one.
                    def f():
                        ps = trans_ps.tile([128, 1024], bf16)
                        for m in range(8):
                            nc.tensor.transpose(
                                ps[:, 128 * m : 128 * (m + 1)],
                                xbfs[8 * tt8 + m][:, 128 * cc : 128 * (cc + 1)],
                                ident,
                            )
                        sl = slice(1024 * tt8, 1024 * (tt8 + 1))
                        nc.vector.tensor_copy(out=xts[cc][:, sl], in_=ps)
                        nc.vector.tensor_copy(out=x8[:, cc, sl], in_=xts[cc][:, sl])
                    return f

                def qk_unit(w8, dst, pair):
                    # two s4 slices share each DoubleRow weight load (the
                    # 256-col LDWEIGHTS is FWL-less and otherwise exposed)
                    def f():
                        s4a, s4b = 2 * pair, 2 * pair + 1
                        psA = mm_ps.tile([128, 512], f32, tag="mm", name="psA")
                        psB = mm_ps.tile([128, 512], f32, tag="mm", name="psB")
                        for g in range(4):
                            for ps, s4 in ((psA, s4a), (psB, s4b)):
                                nc.tensor.matmul(
                                    ps,
                                    w8[:, 2 * g : 2 * g + 2, :],
                                    x8[
                                        :,
                                        2 * g : 2 * g + 2,
                                        512 * s4 : 512 * (s4 + 1),
                                    ],
                                    start=(g == 0),
                                    stop=(g == 3),
                                    perf_mode=DR,
                                )
                        nc.scalar.copy(out=dst[:, 512 * s4a : 512 * (s4a + 1)], in_=psA)
                        nc.scalar.copy(out=dst[:, 512 * s4b : 512 * (s4b + 1)], in_=psB)
                    return f

                def v_unit(s4):
                    def f():
                        ps = mm_ps.tile([128, 512], f32, tag="mm", name="psv")
                        for cc in range(8):
                            nc.tensor.matmul(
                                ps,
                                wtv[:, cc, :],
                                xts[cc][:, 512 * s4 : 512 * (s4 + 1)],
                                start=(cc == 0),
                                stop=(cc == 7),
                            )
                        nc.scalar.copy(out=vT[:, 512 * s4 : 512 * (s4 + 1)], in_=ps)
                    return f

                def va_unit():
                    def f():
                        nc.gpsimd.memset(va[:, :, 128:129], 1.0)
                        nc.sync.dma_start(
                            out=va[:, :, 0:128], in_=vT, transpose=True
                        )
                    return f

                for tt8 in range(2):
                    for cc in range(8):
                        units.append(trans_unit(tt8, cc))
                for pair in range(2):
                    units.append(qk_unit(W8["q"], qT, pair))
                    units.append(qk_unit(W8["k"], kT, pair))
                    units.append(v_unit(2 * pair))
                    units.append(v_unit(2 * pair + 1))
                units.append(va_unit())
                state = dict(qT=qT, kT=kT, va=va)
                return units, state

            def emit_units(units, n):
                for _ in range(n):
                    if units:
                        units.pop(0)()

            def emit_scores(b, st, next_units):
                qT, kT, va = st["qT"], st["kT"], st["va"]
                out_sb = osb_p.tile([128, 16 * H], f32)
                prows = []
                for ss in range(16):
                    pb = 128 * ss
                    pr = pr_p.tile(
                        [128, T - pb], bf16, tag=f"pr{ss}", name=f"pr{ss}"
                    )
                    prows.append(pr)
                    for tq in range(ss // 4, 4):
                        c0 = 512 * tq
                        x0 = max(pb, c0)  # first causal-needed column
                        d0 = x0 - c0
                        sh = srow_ps.tile([128, 512], f32)
                        nc.tensor.matmul(
                            sh[:, d0:512],
                            kT[:, pb : pb + 128],
                            qT[:, x0 : c0 + 512],
                            start=True,
                            stop=True,
                        )
                        nc.scalar.activation(
                            out=pr[:, x0 - pb : c0 + 512 - pb],
                            in_=sh[:, d0:512],
                            func=Exp,
                            scale=EXPSCALE,
                        )
                    # trimask only needs the first (diagonal) exp chunk, so
                    # emit it before the fill units to keep the DVE queue
                    # from delaying P.V's last matmul
                    nc.vector.tensor_mul(pr[:, 0:128], pr[:, 0:128], trimask)
                    # fill the PE queue with next-batch work while the ACT
                    # exp chain for this row drains (skip early rows: the
                    # next batch's x tiles are still loading)
                    if ss >= 5:
                        emit_units(next_units, 3)
                    pv = pv_ps.tile([128, H + 1], f32)
                    for j in range(ss + 1):
                        nc.tensor.matmul(
                            pv,
                            prows[j][:, pb - 128 * j : pb - 128 * j + 128],
                            va[:, j, 0 : H + 1],
                            start=(j == 0),
                            stop=(j == ss),
                        )
                    rc = rc_p.tile([128, 1], f32)
                    nc.vector.reciprocal(rc, pv[:, 128:129])
                    nc.vector.tensor_mul(
                        out_sb[:, H * ss : H * (ss + 1)],
                        pv[:, 0:128],
                        rc.broadcast_to([128, H]),
                    )
                emit_units(next_units, len(next_units))
                # out_sb[p, (g h)] -> out[b, 128g+p, h]; split DMAs so the
                # final transfer after the last normalize is small
                np_split = 4 if b == BL - 1 else 2
                npc = 2048 // np_split
                for hh in range(np_split):
                    nc.sync.dma_start(
                        out=out_ap[b, npc * hh : npc * (hh + 1), :].rearrange(
                            "(g p) h -> p g h", p=128
                        ),
                        in_=out_sb[
                            :, npc // 128 * H * hh : npc // 128 * H * (hh + 1)
                        ].rearrange("p (g h) -> p g h", h=H),
                    )

            # --- software-pipelined batch loop ---
            xbfs = emit_loads(0)
            units, st = make_b_units(0, xbfs)
            emit_units(units, len(units))  # prologue: batch 0 B-phase flat
            for b in range(BL):
                if b + 1 < BL:
                    xbfs = emit_loads(b + 1)
                    next_units, next_st = make_b_units(b + 1, xbfs)
                else:
                    next_units, next_st = [], None
                emit_scores(b, st, next_units)
                st = next_st

    nc.compile()
    return nc


def _get_nc():
    if "nc" not in _CACHE:
        _CACHE["nc"] = _build()
    return _CACHE["nc"]


def kernel(x, Wk, Wq, Wv, _trace=False):
    from concourse.bass_utils import run_bass_kernel_spmd

    x = np.ascontiguousarray(np.asarray(x, dtype=np.float32))
    Wk = np.ascontiguousarray(np.asarray(Wk, dtype=np.float32))
    Wq = np.ascontiguousarray(np.asarray(Wq, dtype=np.float32))
    Wv = np.ascontiguousarray(np.asarray(Wv, dtype=np.float32))
    assert x.shape == (B, T, C)

    nc = _get_nc()
    in_maps = [
        {"x": x[i * BL : (i + 1) * BL], "Wk": Wk, "Wq": Wq, "Wv": Wv}
        for i in range(NCORES)
    ]
    res = run_bass_kernel_spmd(nc, in_maps, list(range(NCORES)), trace=_trace)
    out = np.concatenate([res.results[i]["out"] for i in range(NCORES)], axis=0)
    if _trace:
        _CACHE["last_results"] = res
    return out


# revision 22
# speedup vs baseline: 1.8188x; 1.0790x over previous
"""Single-head causal attention (CustomHead) on 8 Trainium2 NeuronCores.

Reference (per batch b):
    q = x Wq^T ; k = x Wk^T ; v = x Wv^T          (x: [T, C], W*: [H, C])
    S = q k^T * C**-0.5 ; causal mask ; softmax ; out = P v    ([T, H])

Sharding: data-parallel over batch B=32 across 8 cores (4 batches/core).
Each core holds full Wq/Wk/Wv.

Kernel plan per core (T=2048, C=1024, H=128), all bf16 matmuls, fp32 accum:
  - x loaded with SWDGE cast-DMA (fp32->bf16 in the DMA engine).
  - PE-transpose x into x^T (projections contract over C, which must sit
    on the partition dim); DVE copies PSUM->SBUF.
  - q^T/k^T/v^T = W @ x^T with W pre-transposed via one DMA-XBAR
    transpose each at setup.
  - v^T -> natural [s, h] + ones column via four XBAR transposes per
    batch into a strided [128, 16, 144] tile (ones col at 128 makes
    P^T @ [v | 1] accumulate numerator and softmax denominator
    together).
  - Scores computed transposed: S^T[s, t] = kT(s-block) vs qT, 512-wide
    PSUM chunks, one exp (ACT) per chunk; no max-subtraction (scores
    are bounded, exp is safe in fp32).  P^T rows stored bf16 in
    right-sized tiles ([128, 2048-128*ss]).
  - Causal handling: S^T block-row ss only computes t >= 512*(ss//4);
    the diagonal 128x128 block is masked by an upper-triangular 0/1
    multiply after exp; nothing below is ever read.
  - The batch loop is software-pipelined: batch b+1's transpose and
    projection work is emitted in program order between batch b's
    score-row S matmuls and P.V matmuls, so the PE FIFO has useful
    work while the ACT exp chain drains.  Unit order is load-aware:
    projections over the first x^T half run between the two transpose
    half-passes, giving the second half's x tiles time to arrive.
  - Batch 0 uses per-x-tile transposes so the PE starts as soon as the
    first 128-row tile lands instead of waiting for eight.
"""

import numpy as np

B, T, C, H = 32, 2048, 1024, 128
NCORES = 8
BL = B // NCORES  # batches per core

_CACHE = {}


def _build():
    import concourse.bass as bass
    import concourse.tile as tile
    from concourse import bacc, mybir
    from concourse.masks import make_identity, make_upper_triangular

    f32 = mybir.dt.float32
    bf16 = mybir.dt.bfloat16
    Exp = mybir.ActivationFunctionType.Exp
    SC = float(C) ** -0.5

    nc = bacc.Bacc(
        "TRN2",
        target_bir_lowering=False,
        debug=False,
        enable_asserts=False,
        num_devices=NCORES,
    )
    x_ap = nc.dram_tensor("x", [BL, T, C], f32, kind="ExternalInput").ap()
    wk_ap = nc.dram_tensor("Wk", [H, C], f32, kind="ExternalInput").ap()
    wq_ap = nc.dram_tensor("Wq", [H, C], f32, kind="ExternalInput").ap()
    wv_ap = nc.dram_tensor("Wv", [H, C], f32, kind="ExternalInput").ap()
    out_ap = nc.dram_tensor("out", [BL, T, H], f32, kind="ExternalOutput").ap()

    with tile.TileContext(nc) as tc:
        from contextlib import ExitStack

        with ExitStack() as ctx:
            consts = ctx.enter_context(tc.tile_pool(name="consts", bufs=1))
            wstage = ctx.enter_context(tc.tile_pool(name="wstage", bufs=1))
            xbf_p = ctx.enter_context(tc.tile_pool(name="xbf", bufs=16))
            xt_p = ctx.enter_context(tc.tile_pool(name="xt", bufs=9))
            qk_p = ctx.enter_context(tc.tile_pool(name="qk", bufs=2))
            va_p = ctx.enter_context(tc.tile_pool(name="va", bufs=2))
            pr_p = ctx.enter_context(tc.tile_pool(name="prow", bufs=1))
            osb_p = ctx.enter_context(tc.tile_pool(name="osb", bufs=2))
            rc_p = ctx.enter_context(tc.tile_pool(name="rc", bufs=4))
            trans_ps = ctx.enter_context(
                tc.tile_pool(name="trans_ps", bufs=2, space="PSUM")
            )
            mm_ps = ctx.enter_context(tc.tile_pool(name="mm_ps", bufs=2, space="PSUM"))
            srow_ps = ctx.enter_context(
                tc.tile_pool(name="srow_ps", bufs=2, space="PSUM")
            )
            pv_ps = ctx.enter_context(tc.tile_pool(name="pv_ps", bufs=2, space="PSUM"))

            ident = consts.tile([128, 128], bf16)
            make_identity(nc, ident)

            # trimask[s, t] = 1 if s <= t else 0 (valid region of the
            # transposed diagonal block)
            trimask = consts.tile([128, 128], bf16)
            make_upper_triangular(nc, trimask, val=1.0, diag=True)

            # --- weights: load, cast, XBAR-transpose to [c%128, cc, h] ---
            WT = {}
            for name, wap in (("q", wq_ap), ("k", wk_ap), ("v", wv_ap)):
                wnat = wstage.tile([128, C], f32, tag="wnat")
                nc.sync.dma_start(out=wnat, in_=wap)
                wbf = wstage.tile([128, C], bf16, tag="wbf")
                nc.vector.tensor_copy(out=wbf, in_=wnat)
                wt3 = consts.tile([128, 8, 128], bf16, tag=f"wt{name}", name=f"wt{name}")
                nc.sync.dma_start(out=wt3, in_=wbf, transpose=True)
                WT[name] = wt3

            def emit_loads(b):
                xbfs = []
                for tt in range(16):
                    xb = xbf_p.tile([128, C], bf16, tag="xb", name=f"xb{tt}")
                    nc.gpsimd.dma_start(
                        out=xb, in_=x_ap[b, 128 * tt : 128 * (tt + 1), :]
                    )
                    xbfs.append(xb)
                return xbfs

            def make_b_units(b, xbfs, fine_prologue=False):
                """Per-batch transpose/projection work as a list of closures;
                each one is a PE-queue-sized unit.  Order is load-aware."""
                xts = [
                    xt_p.tile([128, T], bf16, name=f"xt{cc}", tag="xt")
                    for cc in range(8)
                ]
                qT = qk_p.tile([128, T], bf16, tag="qT")
                kT = qk_p.tile([128, T], bf16, tag="kT")
                vT = qk_p.tile([128, T], bf16, tag="vT")
                va = va_p.tile([128, 16, 144], bf16)

                def trans_unit(tt8, cc):
                    # one [128, 1024] strip of x^T chunk cc via 8 PE
                    # transposes + 1 DVE copy
                    def f():
                        ps = trans_ps.tile([128, 1024], bf16)
                        for m in range(8):
                            nc.tensor.transpose(
                                ps[:, 128 * m : 128 * (m + 1)],
                                xbfs[8 * tt8 + m][:, 128 * cc : 128 * (cc + 1)],
                                ident,
                            )
                        nc.vector.tensor_copy(
                            out=xts[cc][:, 1024 * tt8 : 1024 * (tt8 + 1)], in_=ps
                        )
                    return f

                def fine_trans_unit(tt):
                    # all 8 chunks of ONE x tile (batch-0 prologue: starts
                    # as soon as each 128-row tile lands)
                    def f():
                        ps = trans_ps.tile([128, 1024], bf16)
                        for cc in range(8):
                            nc.tensor.transpose(
                                ps[:, 128 * cc : 128 * (cc + 1)],
                                xbfs[tt][:, 128 * cc : 128 * (cc + 1)],
                                ident,
                            )
                        for cc in range(8):
                            nc.vector.tensor_copy(
                                out=xts[cc][:, 128 * tt : 128 * (tt + 1)],
                                in_=ps[:, 128 * cc : 128 * (cc + 1)],
                            )
                    return f

                def proj_unit(wt3, dst, s4, with_va=False):
                    def f():
                        ps = mm_ps.tile([128, 512], f32, tag="mm", name="psp")
                        for cc in range(8):
                            nc.tensor.matmul(
                                ps,
                                wt3[:, cc, :],
                                xts[cc][:, 512 * s4 : 512 * (s4 + 1)],
                                start=(cc == 0),
                                stop=(cc == 7),
                            )
                        nc.scalar.copy(out=dst[:, 512 * s4 : 512 * (s4 + 1)], in_=ps)
                        if with_va:
                            if s4 == 0:
                                nc.gpsimd.memset(va[:, :, 128:129], 1.0)
                            nc.sync.dma_start(
                                out=va[:, 4 * s4 : 4 * s4 + 4, 0:128],
                                in_=vT[:, 512 * s4 : 512 * (s4 + 1)],
                                transpose=True,
                            )
                    return f

                units = []
                if fine_prologue:
                    for tt in range(16):
                        units.append(fine_trans_unit(tt))
                    for s4 in range(4):
                        units.append(proj_unit(WT["q"], qT, s4))
                        units.append(proj_unit(WT["k"], kT, s4))
                        units.append(proj_unit(WT["v"], vT, s4, with_va=True))
                else:
                    for cc in range(8):
                        units.append(trans_unit(0, cc))
                    for s4 in (0, 1):
                        units.append(proj_unit(WT["q"], qT, s4))
                        units.append(proj_unit(WT["k"], kT, s4))
                        units.append(proj_unit(WT["v"], vT, s4, with_va=True))
                    for cc in range(8):
                        units.append(trans_unit(1, cc))
                    for s4 in (2, 3):
                        units.append(proj_unit(WT["q"], qT, s4))
                        units.append(proj_unit(WT["k"], kT, s4))
                        units.append(proj_unit(WT["v"], vT, s4, with_va=True))
                state = dict(qT=qT, kT=kT, va=va)
                return units, state

            def emit_units(units, n):
                for _ in range(n):
                    if units:
                        units.pop(0)()

            def emit_scores(b, st, next_units):
                qT, kT, va = st["qT"], st["kT"], st["va"]
                out_sb = osb_p.tile([128, 16 * H], f32)
                prows = []
                for ss in range(16):
                    pb = 128 * ss
                    pr = pr_p.tile(
                        [128, T - pb],
                        bf16,
                        tag=f"pr{ss}",
                        name=f"pr{ss}",
                        bufs=2 if ss < 4 else 1,
                    )
                    prows.append(pr)
                    for tq in range(ss // 4, 4):
                        c0 = 512 * tq
                        x0 = max(pb, c0)  # first causal-needed column
                        d0 = x0 - c0
                        sh = srow_ps.tile([128, 512], f32)
                        nc.tensor.matmul(
                            sh[:, d0:512],
                            kT[:, pb : pb + 128],
                            qT[:, x0 : c0 + 512],
                            start=True,
                            stop=True,
                        )
                        nc.scalar.activation(
                            out=pr[:, x0 - pb : c0 + 512 - pb],
                            in_=sh[:, d0:512],
                            func=Exp,
                            scale=SC,
                        )
                    # trimask only needs the first (diagonal) exp chunk;
                    # emit before the fill units so the DVE queue cannot
                    # delay P.V's final matmul
                    nc.vector.tensor_mul(pr[:, 0:128], pr[:, 0:128], trimask)
                    # fill the PE queue with next-batch work while the ACT
                    # exp chain for this row drains (skip early rows: the
                    # next batch's x tiles are still loading)
                    if ss >= 4:
                        emit_units(next_units, 3)
                    pv = pv_ps.tile([128, H + 1], f32)
                    for j in range(ss + 1):
                        nc.tensor.matmul(
                            pv,
                            prows[j][:, pb - 128 * j : pb - 128 * j + 128],
                            va[:, j, 0 : H + 1],
                            start=(j == 0),
                            stop=(j == ss),
                        )
                    rc = rc_p.tile([128, 1], f32)
                    nc.vector.reciprocal(rc, pv[:, 128:129])
                    nc.vector.tensor_mul(
                        out_sb[:, H * ss : H * (ss + 1)],
                        pv[:, 0:128],
                        rc.broadcast_to([128, H]),
                    )
                emit_units(next_units, len(next_units))
                # out_sb[p, (g h)] -> out[b, 128g+p, h]; split DMAs so the
                # final transfer after the last normalize is small
                np_split = 4 if b == BL - 1 else 2
                npc = 2048 // np_split
                for hh in range(np_split):
                    nc.sync.dma_start(
                        out=out_ap[b, npc * hh : npc * (hh + 1), :].rearrange(
                            "(g p) h -> p g h", p=128
                        ),
                        in_=out_sb[
                            :, npc // 128 * H * hh : npc // 128 * H * (hh + 1)
                        ].rearrange("p (g h) -> p g h", h=H),
                    )

            # --- software-pipelined batch loop ---
            xbfs = emit_loads(0)
            units, st = make_b_units(0, xbfs, fine_prologue=True)
            emit_units(units, len(units))  # prologue: batch 0 B-phase flat
            for b in range(BL):
                if b + 1 < BL:
                    xbfs = emit_loads(b + 1)
                    next_units, next_st = make_b_units(b + 1, xbfs)
                else:
                    next_units, next_st = [], None
                emit_scores(b, st, next_units)
                st = next_st

    nc.compile()
    return nc


def _get_nc():
    if "nc" not in _CACHE:
        _CACHE["nc"] = _build()
    return _CACHE["nc"]


def kernel(x, Wk, Wq, Wv, _trace=False):
    from concourse.bass_utils import run_bass_kernel_spmd

    x = np.ascontiguousarray(np.asarray(x, dtype=np.float32))
    Wk = np.ascontiguousarray(np.asarray(Wk, dtype=np.float32))
    Wq = np.ascontiguousarray(np.asarray(Wq, dtype=np.float32))
    Wv = np.ascontiguousarray(np.asarray(Wv, dtype=np.float32))
    assert x.shape == (B, T, C)

    nc = _get_nc()
    in_maps = [
        {"x": x[i * BL : (i + 1) * BL], "Wk": Wk, "Wq": Wq, "Wv": Wv}
        for i in range(NCORES)
    ]
    res = run_bass_kernel_spmd(nc, in_maps, list(range(NCORES)), trace=_trace)
    out = np.concatenate([res.results[i]["out"] for i in range(NCORES)], axis=0)
    if _trace:
        _CACHE["last_results"] = res
    return out
